# revision 27
# baseline (speedup 1.0000x reference)
"""BEV-pool (segment-sum scatter) Trainium2 kernel for nn_BaseDepthTransform.

Design:
  Host (numpy): replicate the reference geometry -> per-point flat BEV bin id
  (depends only on the small camera matrices, not on x). Sort points by bin.
  Greedily cut the sorted stream into "groups": up to KA*128 points spanning
  < W=16 distinct bins, each group = up to KA=8 point-tiles of 128. Binary-
  decompose group tile-counts into classes {8,4,2,1} so every class has a
  uniform static schedule. Ship, per core: a bf16 feature stream, a bf16
  per-tile one-hot stream ([128 points x 16 bins], built on host), laid out
  in DMA-friendly chunks.

  Device (Bass/Tile, SPMD x8): per group, chain c matmuls
  (one-hot^T @ feats) accumulating the group's [16,80] segment sums in PSUM,
  copy PSUM->SBUF on the Scalar engine, DMA to a per-group output slot.
  Only PE + ACT + DMA are used; no dynamic addressing, no collectives.

  Host reassembly: out[group] is added into grid[base:base+16] (groups may
  share bins across classes/cores; addition commutes).
"""
import sys
sys.path.insert(0, '/opt/trn_rl_repo')

import numpy as np
import ml_dtypes

BF16 = ml_dtypes.bfloat16

# ---- static problem config (mirrors the reference) ----
IH, IW = 256, 704
FH, FW = 32, 88
D = 118
C = 80
NXg, NYg, NZg = 360, 360, 1
BXc = np.array([-53.85, -53.85, 0.0], np.float32)
DXc = np.array([0.3, 0.3, 20.0], np.float32)
NBINS = NZg * NXg * NYg  # 129600
W = 12                   # bins per group window (arbitrary base)
KA = 8                   # max tiles per group / PSUM chain
NCORES = 8
CLASSES = (8, 4, 2, 1)
# groups per DMA chunk / PSUM wave, per class (24 slots = 4 banks, 12 = 2)
CHUNK_GROUPS = {8: 24, 4: 12, 2: 12, 1: 12}  # groups per DMA chunk
PSUM_SLOTS_PER_BANK = 6  # 6 x 80 f32 = 480 of 512
WAVE = 12                # groups per PSUM wave (2 banks)

_BUILD_CACHE = {}


def _frustum():
    ds = np.arange(1.0, 60.0, 0.5, dtype=np.float32)
    xs = np.linspace(0.0, IW - 1.0, FW, dtype=np.float32)
    ys = np.linspace(0.0, IH - 1.0, FH, dtype=np.float32)
    ds_g = np.broadcast_to(ds[:, None, None], (D, FH, FW))
    xs_g = np.broadcast_to(xs[None, None, :], (D, FH, FW))
    ys_g = np.broadcast_to(ys[None, :, None], (D, FH, FW))
    return np.stack([xs_g, ys_g, ds_g], axis=-1)  # [D,FH,FW,3]


def _get_geometry(c2l_rots, c2l_trans, intrins, post_rots, post_trans,
                  extra_rots, extra_trans):
    fr = _frustum()
    pts = fr[None, None] - post_trans[:, :, None, None, None, :]
    inv_pr = np.linalg.inv(post_rots).astype(np.float32)
    pts = np.einsum('bnij,bndhwj->bndhwi', inv_pr, pts).astype(np.float32)
    pts = np.concatenate([pts[..., :2] * pts[..., 2:3], pts[..., 2:3]], axis=-1)
    combine = np.einsum(
        'bnij,bnjk->bnik', c2l_rots, np.linalg.inv(intrins).astype(np.float32)
    ).astype(np.float32)
    pts = np.einsum('bnij,bndhwj->bndhwi', combine, pts).astype(np.float32)
    pts = pts + c2l_trans[:, :, None, None, None, :]
    pts = np.einsum('bij,bndhwj->bndhwi', extra_rots, pts).astype(np.float32)
    pts = pts + extra_trans[:, None, None, None, None, :]
    return pts  # [B,N,D,FH,FW,3]


def _flat_bins(geom):
    """Per-point flat bin id (int64), -1 for dropped points."""
    coords = ((geom - (BXc - DXc / 2.0)) / DXc).astype(np.int32)
    B = coords.shape[0]
    coords = coords.reshape(B, -1, 3)
    cx, cy, cz = coords[..., 0], coords[..., 1], coords[..., 2]
    kept = (cx >= 0) & (cx < NXg) & (cy >= 0) & (cy < NYg) & (cz >= 0) & (cz < NZg)
    flat = ((cz.astype(np.int64) * NXg + cx) * NYg + cy)
    flat = np.where(kept, flat, -1)
    return flat  # [B, Np]


def _round_up(x, m):
    return ((x + m - 1) // m) * m


def _cut_groups(fk_sorted):
    """Greedy: groups of <=KA*128 points spanning < W bins, binary-decomposed
    into class segments [(cls, start, npts, base), ...] in stream order."""
    n = len(fk_sorted)
    segs = []
    i = 0
    while i < n:
        hi = np.searchsorted(fk_sorted, fk_sorted[i] + W, side='left')
        j = min(i + KA * 128, hi, n)
        npts = j - i
        base = int(fk_sorted[i])
        nt = (npts + 127) // 128
        s = i
        for c in CLASSES:
            while nt >= c:
                ln = min(c * 128, j - s)
                segs.append((c, s, ln, base))
                s += ln
                nt -= c
        i = j
    return segs


def _split_classes(segs):
    """Per class: contiguous split across cores balanced by group count,
    padded to uniform per-class counts. {cls: (percore seg lists, Gmax)}."""
    out = {}
    for c in CLASSES:
        cl = [s for s in segs if s[0] == c]
        G = len(cl)
        per = []
        for ci in range(NCORES):
            lo = (G * ci) // NCORES
            hi = (G * (ci + 1)) // NCORES
            per.append(cl[lo:hi])
        Gmax = max(1, max(len(p) for p in per))
        Gmax = _round_up(Gmax, CHUNK_GROUPS[c])
        out[c] = (per, Gmax)
    return out


def _build_core_inputs(class_split, fk_sorted, pidx_sorted, xflat_bf):
    """Build per-core input dict: per class feats + onehot streams."""
    maps = [dict() for _ in range(NCORES)]
    meta = {c: [] for c in CLASSES}  # per class: percore array of bases
    for c in CLASSES:
        per, Gmax = class_split[c]
        T = Gmax * c
        for ci in range(NCORES):
            segs = per[ci]
            feats = np.zeros((T, 128, C), BF16)
            oh = np.zeros((T, 128, W), BF16)
            bases = np.full((Gmax,), -1, np.int64)
            for gi, (_, s, ln, base) in enumerate(segs):
                bases[gi] = base
                lids = (fk_sorted[s:s + ln] - base).astype(np.int64)
                pix = pidx_sorted[s:s + ln]
                t0 = gi * c
                nt = (ln + 127) // 128
                for k in range(nt):
                    a, b = k * 128, min((k + 1) * 128, ln)
                    m = b - a
                    feats[t0 + k, :m] = xflat_bf[pix[a:b]]
                    oh[t0 + k, np.arange(m), lids[a:b]] = 1
            CH = CHUNK_GROUPS[c]
            nch = Gmax // CH
            f = feats.reshape(nch, CH * c, 128, C).transpose(0, 2, 1, 3) \
                     .reshape(nch, 128, CH * c * C)
            o8 = oh.astype(ml_dtypes.float8_e4m3)
            o = o8.reshape(nch, CH * c, 128, W).transpose(0, 2, 1, 3) \
                  .reshape(nch, 128, CH * c * W)
            maps[ci][f"feats{c}"] = np.ascontiguousarray(f)
            maps[ci][f"oh{c}"] = np.ascontiguousarray(o)
            meta[c].append(bases)
    return maps, meta


def _build_bass(shape_key):
    """shape_key: tuple of (cls, Gmax) pairs."""
    if shape_key in _BUILD_CACHE:
        return _BUILD_CACHE[shape_key]
    from concourse import bass, mybir, tile, bacc

    nc = bacc.Bacc()
    params = {}
    for c, Gmax in shape_key:
        CH = CHUNK_GROUPS[c]
        nch = Gmax // CH
        params[f"feats{c}"] = nc.declare_dram_parameter(
            f"feats{c}", [nch, 128, CH * c * C], mybir.dt.bfloat16,
            isOutput=False)
        params[f"oh{c}"] = nc.declare_dram_parameter(
            f"oh{c}", [nch, 128, CH * c * W], mybir.dt.float8e4,
            isOutput=False)
        params[f"out{c}"] = nc.declare_dram_parameter(
            f"out{c}", [W, Gmax, C], mybir.dt.float32, isOutput=True)

    def slot_off(s):
        return (s // PSUM_SLOTS_PER_BANK) * 512 + (s % PSUM_SLOTS_PER_BANK) * C

    # interleave class chunks so short small-class pipelines hide under the
    # dense class-8 stream
    chunk_order = []
    for c, Gmax in shape_key:
        nch = Gmax // CHUNK_GROUPS[c]
        for ch in range(nch):
            chunk_order.append((c, Gmax, ch, (ch + 0.5) / nch))
    chunk_order.sort(key=lambda t: t[3])

    with tile.TileContext(nc) as tc:
        with tc.tile_pool(name="fstream", bufs=5) as fpool, \
             tc.tile_pool(name="stage", bufs=6) as spool, \
             tc.tile_pool(name="psum", bufs=4, space="PSUM") as psum_pool:
            for c, Gmax, ch, _frac in chunk_order:
                CH = CHUNK_GROUPS[c]
                nch = Gmax // CH
                fchunk = fpool.tile([128, CH * c * C], mybir.dt.bfloat16,
                                    tag="fchunk")
                nc.sync.dma_start(fchunk[:], params[f"feats{c}"][ch, :, :])
                ochunk = fpool.tile([128, CH * c * W], mybir.dt.float8e4,
                                    tag="ochunk")
                nc.scalar.dma_start(ochunk[:], params[f"oh{c}"][ch, :, :])
                nwave = (CH + WAVE - 1) // WAVE
                for wv in range(nwave):
                    g0 = wv * WAVE
                    NW = min(WAVE, CH - g0)
                    nbank = NW // PSUM_SLOTS_PER_BANK
                    mega = psum_pool.tile([W, nbank * 512], mybir.dt.float32,
                                          tag="ps")
                    half = NW // 2
                    for gp in range(half):
                        ga, gb = g0 + gp, g0 + gp + half
                        oa, ob = slot_off(gp), slot_off(gp + half)
                        for k in range(c):
                            ta = ga * c + k
                            tb = gb * c + k
                            nc.tensor.matmul(
                                out=mega[:, oa:oa + C],
                                lhsT=ochunk[:, ta * W:(ta + 1) * W],
                                rhs=fchunk[:, ta * C:(ta + 1) * C],
                                start=(k == 0), stop=(k == c - 1))
                            nc.tensor.matmul(
                                out=mega[:, ob:ob + C],
                                lhsT=ochunk[:, tb * W:(tb + 1) * W],
                                rhs=fchunk[:, tb * C:(tb + 1) * C],
                                start=(k == 0), stop=(k == c - 1))
                    st = spool.tile([W, NW, C], mybir.dt.float32, tag="st")
                    src_ap = bass.AP(
                        mega[:].tensor, mega[:].offset,
                        [mega[:].ap[0], [512, nbank],
                         [C, PSUM_SLOTS_PER_BANK], [1, C]])
                    dst_ap = bass.AP(
                        st[:].tensor, st[:].offset,
                        [st[:].ap[0], [PSUM_SLOTS_PER_BANK * C, nbank],
                         [C, PSUM_SLOTS_PER_BANK], [1, C]])
                    nc.scalar.copy(dst_ap, src_ap)
                    nc.scalar.dma_start(
                        params[f"out{c}"][:, ch * CH + g0:ch * CH + g0 + NW, :],
                        st[:])
    nc.finalize()
    _BUILD_CACHE[shape_key] = nc
    return nc


def run_scheduled(x, flat, trace=False, trace_cores=None):
    """Core pipeline given precomputed flat bins; returns (grid, results)."""
    from concourse.bass_utils import run_bass_kernel_spmd

    xflat_bf = np.ascontiguousarray(x.reshape(-1, C)).astype(BF16)
    kept_idx = np.nonzero(flat >= 0)[0]
    fk = flat[kept_idx]
    order = np.argsort(fk, kind='stable')
    fk_sorted = fk[order]
    pidx_sorted = kept_idx[order]

    segs = _cut_groups(fk_sorted)
    class_split = _split_classes(segs)
    shape_key = tuple((c, class_split[c][1]) for c in CLASSES)

    maps, meta = _build_core_inputs(class_split, fk_sorted, pidx_sorted,
                                    xflat_bf)
    nc = _build_bass(shape_key)
    res = run_bass_kernel_spmd(nc, maps, core_ids=list(range(NCORES)),
                               trace=trace, trace_cores=trace_cores)

    grid = np.zeros((NBINS + W, C), np.float32)
    for c in CLASSES:
        for ci in range(NCORES):
            outs = res.results[ci][f"out{c}"]   # [W, Gmax, C]
            bases = meta[c][ci]
            for gi in range(len(bases)):
                base = bases[gi]
                if base >= 0:
                    grid[base:base + W] += outs[:, gi]
    return grid[:NBINS], res


def kernel(x, camera2lidar_rots, camera2lidar_trans, intrins, post_rots,
           post_trans, extra_rots, extra_trans):
    x = np.asarray(x, np.float32)
    B, N = x.shape[0], x.shape[1]
    assert (B, N) == (1, 6) and x.shape[2:] == (D, FH, FW, C), x.shape

    geom = _get_geometry(
        np.asarray(camera2lidar_rots, np.float32),
        np.asarray(camera2lidar_trans, np.float32),
        np.asarray(intrins, np.float32),
        np.asarray(post_rots, np.float32),
        np.asarray(post_trans, np.float32),
        np.asarray(extra_rots, np.float32),
        np.asarray(extra_trans, np.float32),
    )
    flat = _flat_bins(geom)[0]          # [Np]
    grid, _ = run_scheduled(x, flat)
    outp = grid.reshape(NXg, NYg, C).transpose(2, 0, 1)[None]  # [1,C,NX,NY]
    return np.ascontiguousarray(outp)


# revision 28
# speedup vs baseline: 1.0158x; 1.0158x over previous
"""BEV-pool (segment-sum scatter) Trainium2 kernel for nn_BaseDepthTransform.

Design:
  Host (numpy): replicate the reference geometry -> per-point flat BEV bin id
  (depends only on the small camera matrices, not on x). Sort points by bin.
  Greedily cut the sorted stream into "groups": up to KA*128 points spanning
  < W=16 distinct bins, each group = up to KA=8 point-tiles of 128. Binary-
  decompose group tile-counts into classes {8,4,2,1} so every class has a
  uniform static schedule. Ship, per core: a bf16 feature stream, a bf16
  per-tile one-hot stream ([128 points x 16 bins], built on host), laid out
  in DMA-friendly chunks.

  Device (Bass/Tile, SPMD x8): per group, chain c matmuls
  (one-hot^T @ feats) accumulating the group's [16,80] segment sums in PSUM,
  copy PSUM->SBUF on the Scalar engine, DMA to a per-group output slot.
  Only PE + ACT + DMA are used; no dynamic addressing, no collectives.

  Host reassembly: out[group] is added into grid[base:base+16] (groups may
  share bins across classes/cores; addition commutes).
"""
import sys
sys.path.insert(0, '/opt/trn_rl_repo')

import numpy as np
import ml_dtypes

BF16 = ml_dtypes.bfloat16

# ---- static problem config (mirrors the reference) ----
IH, IW = 256, 704
FH, FW = 32, 88
D = 118
C = 80
NXg, NYg, NZg = 360, 360, 1
BXc = np.array([-53.85, -53.85, 0.0], np.float32)
DXc = np.array([0.3, 0.3, 20.0], np.float32)
NBINS = NZg * NXg * NYg  # 129600
W = 16                   # bins per group window (arbitrary base)
KA = 8                   # max tiles per group / PSUM chain
NCORES = 8
CLASSES = (8, 4, 2, 1)
# groups per DMA chunk / PSUM wave, per class (24 slots = 4 banks, 12 = 2)
CHUNK_GROUPS = {8: 24, 4: 12, 2: 12, 1: 12}  # groups per DMA chunk
PSUM_SLOTS_PER_BANK = 6  # 6 x 80 f32 = 480 of 512
WAVE = 12                # groups per PSUM wave (2 banks)

_BUILD_CACHE = {}


def _frustum():
    ds = np.arange(1.0, 60.0, 0.5, dtype=np.float32)
    xs = np.linspace(0.0, IW - 1.0, FW, dtype=np.float32)
    ys = np.linspace(0.0, IH - 1.0, FH, dtype=np.float32)
    ds_g = np.broadcast_to(ds[:, None, None], (D, FH, FW))
    xs_g = np.broadcast_to(xs[None, None, :], (D, FH, FW))
    ys_g = np.broadcast_to(ys[None, :, None], (D, FH, FW))
    return np.stack([xs_g, ys_g, ds_g], axis=-1)  # [D,FH,FW,3]


def _get_geometry(c2l_rots, c2l_trans, intrins, post_rots, post_trans,
                  extra_rots, extra_trans):
    fr = _frustum()
    pts = fr[None, None] - post_trans[:, :, None, None, None, :]
    inv_pr = np.linalg.inv(post_rots).astype(np.float32)
    pts = np.einsum('bnij,bndhwj->bndhwi', inv_pr, pts).astype(np.float32)
    pts = np.concatenate([pts[..., :2] * pts[..., 2:3], pts[..., 2:3]], axis=-1)
    combine = np.einsum(
        'bnij,bnjk->bnik', c2l_rots, np.linalg.inv(intrins).astype(np.float32)
    ).astype(np.float32)
    pts = np.einsum('bnij,bndhwj->bndhwi', combine, pts).astype(np.float32)
    pts = pts + c2l_trans[:, :, None, None, None, :]
    pts = np.einsum('bij,bndhwj->bndhwi', extra_rots, pts).astype(np.float32)
    pts = pts + extra_trans[:, None, None, None, None, :]
    return pts  # [B,N,D,FH,FW,3]


def _flat_bins(geom):
    """Per-point flat bin id (int64), -1 for dropped points."""
    coords = ((geom - (BXc - DXc / 2.0)) / DXc).astype(np.int32)
    B = coords.shape[0]
    coords = coords.reshape(B, -1, 3)
    cx, cy, cz = coords[..., 0], coords[..., 1], coords[..., 2]
    kept = (cx >= 0) & (cx < NXg) & (cy >= 0) & (cy < NYg) & (cz >= 0) & (cz < NZg)
    flat = ((cz.astype(np.int64) * NXg + cx) * NYg + cy)
    flat = np.where(kept, flat, -1)
    return flat  # [B, Np]


def _round_up(x, m):
    return ((x + m - 1) // m) * m


def _cut_groups(fk_sorted):
    """Greedy: groups of <=KA*128 points spanning < W bins, binary-decomposed
    into class segments [(cls, start, npts, base), ...] in stream order."""
    n = len(fk_sorted)
    segs = []
    i = 0
    while i < n:
        hi = np.searchsorted(fk_sorted, fk_sorted[i] + W, side='left')
        j = min(i + KA * 128, hi, n)
        npts = j - i
        base = int(fk_sorted[i])
        nt = (npts + 127) // 128
        s = i
        for c in CLASSES:
            while nt >= c:
                ln = min(c * 128, j - s)
                segs.append((c, s, ln, base))
                s += ln
                nt -= c
        i = j
    return segs


def _split_classes(segs):
    """Per class: contiguous split across cores balanced by group count,
    padded to uniform per-class counts. {cls: (percore seg lists, Gmax)}."""
    out = {}
    for c in CLASSES:
        cl = [s for s in segs if s[0] == c]
        G = len(cl)
        per = []
        for ci in range(NCORES):
            lo = (G * ci) // NCORES
            hi = (G * (ci + 1)) // NCORES
            per.append(cl[lo:hi])
        Gmax = max(1, max(len(p) for p in per))
        Gmax = _round_up(Gmax, CHUNK_GROUPS[c])
        out[c] = (per, Gmax)
    return out


def _build_core_inputs(class_split, fk_sorted, pidx_sorted, xflat_bf):
    """Build per-core input dict: per class feats + onehot streams."""
    maps = [dict() for _ in range(NCORES)]
    meta = {c: [] for c in CLASSES}  # per class: percore array of bases
    for c in CLASSES:
        per, Gmax = class_split[c]
        T = Gmax * c
        for ci in range(NCORES):
            segs = per[ci]
            feats = np.zeros((T, 128, C), BF16)
            oh = np.zeros((T, 128, W), BF16)
            bases = np.full((Gmax,), -1, np.int64)
            for gi, (_, s, ln, base) in enumerate(segs):
                bases[gi] = base
                lids = (fk_sorted[s:s + ln] - base).astype(np.int64)
                pix = pidx_sorted[s:s + ln]
                t0 = gi * c
                nt = (ln + 127) // 128
                for k in range(nt):
                    a, b = k * 128, min((k + 1) * 128, ln)
                    m = b - a
                    feats[t0 + k, :m] = xflat_bf[pix[a:b]]
                    oh[t0 + k, np.arange(m), lids[a:b]] = 1
            CH = CHUNK_GROUPS[c]
            nch = Gmax // CH
            f = feats.reshape(nch, CH * c, 128, C).transpose(0, 2, 1, 3) \
                     .reshape(nch, 128, CH * c * C)
            o8 = oh.astype(ml_dtypes.float8_e4m3)
            o = o8.reshape(nch, CH * c, 128, W).transpose(0, 2, 1, 3) \
                  .reshape(nch, 128, CH * c * W)
            maps[ci][f"feats{c}"] = np.ascontiguousarray(f)
            maps[ci][f"oh{c}"] = np.ascontiguousarray(o)
            meta[c].append(bases)
    return maps, meta


def _build_bass(shape_key):
    """shape_key: tuple of (cls, Gmax) pairs."""
    if shape_key in _BUILD_CACHE:
        return _BUILD_CACHE[shape_key]
    from concourse import bass, mybir, tile, bacc

    nc = bacc.Bacc()
    params = {}
    for c, Gmax in shape_key:
        CH = CHUNK_GROUPS[c]
        nch = Gmax // CH
        params[f"feats{c}"] = nc.declare_dram_parameter(
            f"feats{c}", [nch, 128, CH * c * C], mybir.dt.bfloat16,
            isOutput=False)
        params[f"oh{c}"] = nc.declare_dram_parameter(
            f"oh{c}", [nch, 128, CH * c * W], mybir.dt.float8e4,
            isOutput=False)
        params[f"out{c}"] = nc.declare_dram_parameter(
            f"out{c}", [W, Gmax, C], mybir.dt.float32, isOutput=True)

    def slot_off(s):
        return (s // PSUM_SLOTS_PER_BANK) * 512 + (s % PSUM_SLOTS_PER_BANK) * C

    # interleave class chunks so short small-class pipelines hide under the
    # dense class-8 stream
    chunk_order = []
    for c, Gmax in shape_key:
        nch = Gmax // CHUNK_GROUPS[c]
        for ch in range(nch):
            chunk_order.append((c, Gmax, ch, (ch + 0.5) / nch))
    chunk_order.sort(key=lambda t: t[3])

    with tile.TileContext(nc) as tc:
        with tc.tile_pool(name="fstream", bufs=5) as fpool, \
             tc.tile_pool(name="stage", bufs=6) as spool, \
             tc.tile_pool(name="psum", bufs=4, space="PSUM") as psum_pool:
            for c, Gmax, ch, _frac in chunk_order:
                CH = CHUNK_GROUPS[c]
                nch = Gmax // CH
                fchunk = fpool.tile([128, CH * c * C], mybir.dt.bfloat16,
                                    tag="fchunk")
                nc.sync.dma_start(fchunk[:], params[f"feats{c}"][ch, :, :])
                ochunk = fpool.tile([128, CH * c * W], mybir.dt.float8e4,
                                    tag="ochunk")
                nc.scalar.dma_start(ochunk[:], params[f"oh{c}"][ch, :, :])
                nwave = (CH + WAVE - 1) // WAVE
                for wv in range(nwave):
                    g0 = wv * WAVE
                    NW = min(WAVE, CH - g0)
                    nbank = NW // PSUM_SLOTS_PER_BANK
                    mega = psum_pool.tile([W, nbank * 512], mybir.dt.float32,
                                          tag="ps")
                    half = NW // 2
                    for gp in range(half):
                        ga, gb = g0 + gp, g0 + gp + half
                        oa, ob = slot_off(gp), slot_off(gp + half)
                        for k in range(c):
                            ta = ga * c + k
                            tb = gb * c + k
                            nc.tensor.matmul(
                                out=mega[:, oa:oa + C],
                                lhsT=ochunk[:, ta * W:(ta + 1) * W],
                                rhs=fchunk[:, ta * C:(ta + 1) * C],
                                start=(k == 0), stop=(k == c - 1))
                            nc.tensor.matmul(
                                out=mega[:, ob:ob + C],
                                lhsT=ochunk[:, tb * W:(tb + 1) * W],
                                rhs=fchunk[:, tb * C:(tb + 1) * C],
                                start=(k == 0), stop=(k == c - 1))
                    st = spool.tile([W, NW, C], mybir.dt.float32, tag="st")
                    src_ap = bass.AP(
                        mega[:].tensor, mega[:].offset,
                        [mega[:].ap[0], [512, nbank],
                         [C, PSUM_SLOTS_PER_BANK], [1, C]])
                    dst_ap = bass.AP(
                        st[:].tensor, st[:].offset,
                        [st[:].ap[0], [PSUM_SLOTS_PER_BANK * C, nbank],
                         [C, PSUM_SLOTS_PER_BANK], [1, C]])
                    nc.scalar.copy(dst_ap, src_ap)
                    nc.scalar.dma_start(
                        params[f"out{c}"][:, ch * CH + g0:ch * CH + g0 + NW, :],
                        st[:])
    nc.finalize()
    _BUILD_CACHE[shape_key] = nc
    return nc


def run_scheduled(x, flat, trace=False, trace_cores=None):
    """Core pipeline given precomputed flat bins; returns (grid, results)."""
    from concourse.bass_utils import run_bass_kernel_spmd

    xflat_bf = np.ascontiguousarray(x.reshape(-1, C)).astype(BF16)
    kept_idx = np.nonzero(flat >= 0)[0]
    fk = flat[kept_idx]
    order = np.argsort(fk, kind='stable')
    fk_sorted = fk[order]
    pidx_sorted = kept_idx[order]

    segs = _cut_groups(fk_sorted)
    class_split = _split_classes(segs)
    shape_key = tuple((c, class_split[c][1]) for c in CLASSES)

    maps, meta = _build_core_inputs(class_split, fk_sorted, pidx_sorted,
                                    xflat_bf)
    nc = _build_bass(shape_key)
    res = run_bass_kernel_spmd(nc, maps, core_ids=list(range(NCORES)),
                               trace=trace, trace_cores=trace_cores)

    grid = np.zeros((NBINS + W, C), np.float32)
    for c in CLASSES:
        for ci in range(NCORES):
            outs = res.results[ci][f"out{c}"]   # [W, Gmax, C]
            bases = meta[c][ci]
            for gi in range(len(bases)):
                base = bases[gi]
                if base >= 0:
                    grid[base:base + W] += outs[:, gi]
    return grid[:NBINS], res


def kernel(x, camera2lidar_rots, camera2lidar_trans, intrins, post_rots,
           post_trans, extra_rots, extra_trans):
    x = np.asarray(x, np.float32)
    B, N = x.shape[0], x.shape[1]
    assert (B, N) == (1, 6) and x.shape[2:] == (D, FH, FW, C), x.shape

    geom = _get_geometry(
        np.asarray(camera2lidar_rots, np.float32),
        np.asarray(camera2lidar_trans, np.float32),
        np.asarray(intrins, np.float32),
        np.asarray(post_rots, np.float32),
        np.asarray(post_trans, np.float32),
        np.asarray(extra_rots, np.float32),
        np.asarray(extra_trans, np.float32),
    )
    flat = _flat_bins(geom)[0]          # [Np]
    grid, _ = run_scheduled(x, flat)
    outp = grid.reshape(NXg, NYg, C).transpose(2, 0, 1)[None]  # [1,C,NX,NY]
    return np.ascontiguousarray(outp)


# revision 29
# speedup vs baseline: 1.0246x; 1.0087x over previous
"""BEV-pool (segment-sum scatter) Trainium2 kernel for nn_BaseDepthTransform.

Design:
  Host (numpy): replicate the reference geometry -> per-point flat BEV bin id
  (depends only on the small camera matrices, not on x). Sort points by bin.
  Greedily cut the sorted stream into "groups": up to KA*128 points spanning
  < W=16 distinct bins, each group = up to KA=8 point-tiles of 128. Binary-
  decompose group tile-counts into classes {8,4,2,1} so every class has a
  uniform static schedule. Ship, per core: a bf16 feature stream, a bf16
  per-tile one-hot stream ([128 points x 16 bins], built on host), laid out
  in DMA-friendly chunks.

  Device (Bass/Tile, SPMD x8): per group, chain c matmuls
  (one-hot^T @ feats) accumulating the group's [16,80] segment sums in PSUM,
  copy PSUM->SBUF on the Scalar engine, DMA to a per-group output slot.
  Only PE + ACT + DMA are used; no dynamic addressing, no collectives.

  Host reassembly: out[group] is added into grid[base:base+16] (groups may
  share bins across classes/cores; addition commutes).
"""
import sys
sys.path.insert(0, '/opt/trn_rl_repo')

import numpy as np
import ml_dtypes

BF16 = ml_dtypes.bfloat16

# ---- static problem config (mirrors the reference) ----
IH, IW = 256, 704
FH, FW = 32, 88
D = 118
C = 80
NXg, NYg, NZg = 360, 360, 1
BXc = np.array([-53.85, -53.85, 0.0], np.float32)
DXc = np.array([0.3, 0.3, 20.0], np.float32)
NBINS = NZg * NXg * NYg  # 129600
W = 16                   # bins per group window (arbitrary base)
KA = 8                   # max tiles per group / PSUM chain
NCORES = 8
CLASSES = (8, 4, 2, 1)
# groups per DMA chunk / PSUM wave, per class (24 slots = 4 banks, 12 = 2)
CHUNK_GROUPS = {8: 24, 4: 12, 2: 12, 1: 12}  # groups per DMA chunk
PSUM_SLOTS_PER_BANK = 6  # 6 x 80 f32 = 480 of 512
WAVE = 12                # groups per PSUM wave (2 banks)

_BUILD_CACHE = {}


def _frustum():
    ds = np.arange(1.0, 60.0, 0.5, dtype=np.float32)
    xs = np.linspace(0.0, IW - 1.0, FW, dtype=np.float32)
    ys = np.linspace(0.0, IH - 1.0, FH, dtype=np.float32)
    ds_g = np.broadcast_to(ds[:, None, None], (D, FH, FW))
    xs_g = np.broadcast_to(xs[None, None, :], (D, FH, FW))
    ys_g = np.broadcast_to(ys[None, :, None], (D, FH, FW))
    return np.stack([xs_g, ys_g, ds_g], axis=-1)  # [D,FH,FW,3]


def _get_geometry(c2l_rots, c2l_trans, intrins, post_rots, post_trans,
                  extra_rots, extra_trans):
    fr = _frustum()
    pts = fr[None, None] - post_trans[:, :, None, None, None, :]
    inv_pr = np.linalg.inv(post_rots).astype(np.float32)
    pts = np.einsum('bnij,bndhwj->bndhwi', inv_pr, pts).astype(np.float32)
    pts = np.concatenate([pts[..., :2] * pts[..., 2:3], pts[..., 2:3]], axis=-1)
    combine = np.einsum(
        'bnij,bnjk->bnik', c2l_rots, np.linalg.inv(intrins).astype(np.float32)
    ).astype(np.float32)
    pts = np.einsum('bnij,bndhwj->bndhwi', combine, pts).astype(np.float32)
    pts = pts + c2l_trans[:, :, None, None, None, :]
    pts = np.einsum('bij,bndhwj->bndhwi', extra_rots, pts).astype(np.float32)
    pts = pts + extra_trans[:, None, None, None, None, :]
    return pts  # [B,N,D,FH,FW,3]


def _flat_bins(geom):
    """Per-point flat bin id (int64), -1 for dropped points."""
    coords = ((geom - (BXc - DXc / 2.0)) / DXc).astype(np.int32)
    B = coords.shape[0]
    coords = coords.reshape(B, -1, 3)
    cx, cy, cz = coords[..., 0], coords[..., 1], coords[..., 2]
    kept = (cx >= 0) & (cx < NXg) & (cy >= 0) & (cy < NYg) & (cz >= 0) & (cz < NZg)
    flat = ((cz.astype(np.int64) * NXg + cx) * NYg + cy)
    flat = np.where(kept, flat, -1)
    return flat  # [B, Np]


def _round_up(x, m):
    return ((x + m - 1) // m) * m


def _cut_groups(fk_sorted):
    """Greedy: groups of <=KA*128 points spanning < W bins, binary-decomposed
    into class segments [(cls, start, npts, base), ...] in stream order."""
    n = len(fk_sorted)
    segs = []
    i = 0
    while i < n:
        hi = np.searchsorted(fk_sorted, fk_sorted[i] + W, side='left')
        j = min(i + KA * 128, hi, n)
        npts = j - i
        base = int(fk_sorted[i])
        nt = (npts + 127) // 128
        s = i
        for c in CLASSES:
            while nt >= c:
                ln = min(c * 128, j - s)
                segs.append((c, s, ln, base))
                s += ln
                nt -= c
        i = j
    return segs


def _split_classes(segs):
    """Per class: contiguous split across cores balanced by group count,
    padded to uniform per-class counts. {cls: (percore seg lists, Gmax)}."""
    out = {}
    for c in CLASSES:
        cl = [s for s in segs if s[0] == c]
        G = len(cl)
        per = []
        for ci in range(NCORES):
            lo = (G * ci) // NCORES
            hi = (G * (ci + 1)) // NCORES
            per.append(cl[lo:hi])
        Gmax = max(1, max(len(p) for p in per))
        Gmax = _round_up(Gmax, CHUNK_GROUPS[c])
        out[c] = (per, Gmax)
    return out


def _build_core_inputs(class_split, fk_sorted, pidx_sorted, xflat_bf):
    """Build per-core input dict: per class feats + onehot streams."""
    maps = [dict() for _ in range(NCORES)]
    meta = {c: [] for c in CLASSES}  # per class: percore array of bases
    for c in CLASSES:
        per, Gmax = class_split[c]
        T = Gmax * c
        for ci in range(NCORES):
            segs = per[ci]
            feats = np.zeros((T, 128, C), BF16)
            oh = np.zeros((T, 128, W), BF16)
            bases = np.full((Gmax,), -1, np.int64)
            for gi, (_, s, ln, base) in enumerate(segs):
                bases[gi] = base
                lids = (fk_sorted[s:s + ln] - base).astype(np.int64)
                pix = pidx_sorted[s:s + ln]
                t0 = gi * c
                nt = (ln + 127) // 128
                for k in range(nt):
                    a, b = k * 128, min((k + 1) * 128, ln)
                    m = b - a
                    feats[t0 + k, :m] = xflat_bf[pix[a:b]]
                    oh[t0 + k, np.arange(m), lids[a:b]] = 1
            CH = CHUNK_GROUPS[c]
            nch = Gmax // CH
            f = feats.reshape(nch, CH * c, 128, C).transpose(0, 2, 1, 3) \
                     .reshape(nch, 128, CH * c * C)
            o8 = oh.astype(ml_dtypes.float8_e4m3)
            o = o8.reshape(nch, CH * c, 128, W).transpose(0, 2, 1, 3) \
                  .reshape(nch, 128, CH * c * W)
            maps[ci][f"feats{c}"] = np.ascontiguousarray(f)
            maps[ci][f"oh{c}"] = np.ascontiguousarray(o)
            meta[c].append(bases)
    return maps, meta


def _build_bass(shape_key):
    """shape_key: tuple of (cls, Gmax) pairs."""
    if shape_key in _BUILD_CACHE:
        return _BUILD_CACHE[shape_key]
    from concourse import bass, mybir, tile, bacc

    nc = bacc.Bacc()
    params = {}
    for c, Gmax in shape_key:
        CH = CHUNK_GROUPS[c]
        nch = Gmax // CH
        params[f"feats{c}"] = nc.declare_dram_parameter(
            f"feats{c}", [nch, 128, CH * c * C], mybir.dt.bfloat16,
            isOutput=False)
        params[f"oh{c}"] = nc.declare_dram_parameter(
            f"oh{c}", [nch, 128, CH * c * W], mybir.dt.float8e4,
            isOutput=False)
        params[f"out{c}"] = nc.declare_dram_parameter(
            f"out{c}", [W, Gmax, C], mybir.dt.float32, isOutput=True)

    def slot_off(s):
        return (s // PSUM_SLOTS_PER_BANK) * 512 + (s % PSUM_SLOTS_PER_BANK) * C

    # interleave class chunks so short small-class pipelines hide under the
    # dense class-8 stream
    chunk_order = []
    for c, Gmax in shape_key:
        nch = Gmax // CHUNK_GROUPS[c]
        for ch in range(nch):
            chunk_order.append((c, Gmax, ch, (ch + 0.5) / nch))
    chunk_order.sort(key=lambda t: t[3])
    # lead with the two smallest chunks so PE starts ~immediately instead of
    # waiting behind multi-MB prefetches
    small_first = [t for t in chunk_order if t[0] <= 2][:2]
    chunk_order = small_first + [t for t in chunk_order if t not in small_first]

    with tile.TileContext(nc) as tc:
        with tc.tile_pool(name="fstream", bufs=5) as fpool, \
             tc.tile_pool(name="stage", bufs=6) as spool, \
             tc.tile_pool(name="psum", bufs=4, space="PSUM") as psum_pool:
            for c, Gmax, ch, _frac in chunk_order:
                CH = CHUNK_GROUPS[c]
                nch = Gmax // CH
                fchunk = fpool.tile([128, CH * c * C], mybir.dt.bfloat16,
                                    tag="fchunk")
                nc.sync.dma_start(fchunk[:], params[f"feats{c}"][ch, :, :])
                ochunk = fpool.tile([128, CH * c * W], mybir.dt.float8e4,
                                    tag="ochunk")
                nc.scalar.dma_start(ochunk[:], params[f"oh{c}"][ch, :, :])
                nwave = (CH + WAVE - 1) // WAVE
                for wv in range(nwave):
                    g0 = wv * WAVE
                    NW = min(WAVE, CH - g0)
                    nbank = NW // PSUM_SLOTS_PER_BANK
                    mega = psum_pool.tile([W, nbank * 512], mybir.dt.float32,
                                          tag="ps")
                    half = NW // 2
                    for gp in range(half):
                        ga, gb = g0 + gp, g0 + gp + half
                        oa, ob = slot_off(gp), slot_off(gp + half)
                        for k in range(c):
                            ta = ga * c + k
                            tb = gb * c + k
                            nc.tensor.matmul(
                                out=mega[:, oa:oa + C],
                                lhsT=ochunk[:, ta * W:(ta + 1) * W],
                                rhs=fchunk[:, ta * C:(ta + 1) * C],
                                start=(k == 0), stop=(k == c - 1))
                            nc.tensor.matmul(
                                out=mega[:, ob:ob + C],
                                lhsT=ochunk[:, tb * W:(tb + 1) * W],
                                rhs=fchunk[:, tb * C:(tb + 1) * C],
                                start=(k == 0), stop=(k == c - 1))
                    st = spool.tile([W, NW, C], mybir.dt.float32, tag="st")
                    src_ap = bass.AP(
                        mega[:].tensor, mega[:].offset,
                        [mega[:].ap[0], [512, nbank],
                         [C, PSUM_SLOTS_PER_BANK], [1, C]])
                    dst_ap = bass.AP(
                        st[:].tensor, st[:].offset,
                        [st[:].ap[0], [PSUM_SLOTS_PER_BANK * C, nbank],
                         [C, PSUM_SLOTS_PER_BANK], [1, C]])
                    nc.scalar.copy(dst_ap, src_ap)
                    nc.scalar.dma_start(
                        params[f"out{c}"][:, ch * CH + g0:ch * CH + g0 + NW, :],
                        st[:])
    nc.finalize()
    _BUILD_CACHE[shape_key] = nc
    return nc


def run_scheduled(x, flat, trace=False, trace_cores=None):
    """Core pipeline given precomputed flat bins; returns (grid, results)."""
    from concourse.bass_utils import run_bass_kernel_spmd

    xflat_bf = np.ascontiguousarray(x.reshape(-1, C)).astype(BF16)
    kept_idx = np.nonzero(flat >= 0)[0]
    fk = flat[kept_idx]
    order = np.argsort(fk, kind='stable')
    fk_sorted = fk[order]
    pidx_sorted = kept_idx[order]

    segs = _cut_groups(fk_sorted)
    class_split = _split_classes(segs)
    shape_key = tuple((c, class_split[c][1]) for c in CLASSES)

    maps, meta = _build_core_inputs(class_split, fk_sorted, pidx_sorted,
                                    xflat_bf)
    nc = _build_bass(shape_key)
    res = run_bass_kernel_spmd(nc, maps, core_ids=list(range(NCORES)),
                               trace=trace, trace_cores=trace_cores)

    grid = np.zeros((NBINS + W, C), np.float32)
    for c in CLASSES:
        for ci in range(NCORES):
            outs = res.results[ci][f"out{c}"]   # [W, Gmax, C]
            bases = meta[c][ci]
            for gi in range(len(bases)):
                base = bases[gi]
                if base >= 0:
                    grid[base:base + W] += outs[:, gi]
    return grid[:NBINS], res


def kernel(x, camera2lidar_rots, camera2lidar_trans, intrins, post_rots,
           post_trans, extra_rots, extra_trans):
    x = np.asarray(x, np.float32)
    B, N = x.shape[0], x.shape[1]
    assert (B, N) == (1, 6) and x.shape[2:] == (D, FH, FW, C), x.shape

    geom = _get_geometry(
        np.asarray(camera2lidar_rots, np.float32),
        np.asarray(camera2lidar_trans, np.float32),
        np.asarray(intrins, np.float32),
        np.asarray(post_rots, np.float32),
        np.asarray(post_trans, np.float32),
        np.asarray(extra_rots, np.float32),
        np.asarray(extra_trans, np.float32),
    )
    flat = _flat_bins(geom)[0]          # [Np]
    grid, _ = run_scheduled(x, flat)
    outp = grid.reshape(NXg, NYg, C).transpose(2, 0, 1)[None]  # [1,C,NX,NY]
    return np.ascontiguousarray(outp)


# revision 30
# speedup vs baseline: 1.0784x; 1.0525x over previous
"""BEV-pool (segment-sum scatter) Trainium2 kernel for nn_BaseDepthTransform.

Design:
  Host (numpy): replicate the reference geometry -> per-point flat BEV bin id
  (depends only on the small camera matrices, not on x). Sort points by bin.
  Greedily cut the sorted stream into "groups": up to KA*128 points spanning
  < W=16 distinct bins, each group = up to KA=8 point-tiles of 128. Binary-
  decompose group tile-counts into classes {8,4,2,1} so every class has a
  uniform static schedule. Ship, per core: a bf16 feature stream, a bf16
  per-tile one-hot stream ([128 points x 16 bins], built on host), laid out
  in DMA-friendly chunks.

  Device (Bass/Tile, SPMD x8): per group, chain c matmuls
  (one-hot^T @ feats) accumulating the group's [16,80] segment sums in PSUM,
  copy PSUM->SBUF on the Scalar engine, DMA to a per-group output slot.
  Only PE + ACT + DMA are used; no dynamic addressing, no collectives.

  Host reassembly: out[group] is added into grid[base:base+16] (groups may
  share bins across classes/cores; addition commutes).
"""
import sys
sys.path.insert(0, '/opt/trn_rl_repo')

import numpy as np
import ml_dtypes

BF16 = ml_dtypes.bfloat16

# ---- static problem config (mirrors the reference) ----
IH, IW = 256, 704
FH, FW = 32, 88
D = 118
C = 80
NXg, NYg, NZg = 360, 360, 1
BXc = np.array([-53.85, -53.85, 0.0], np.float32)
DXc = np.array([0.3, 0.3, 20.0], np.float32)
NBINS = NZg * NXg * NYg  # 129600
W = 16                   # bins per group window (arbitrary base)
KA = 8                   # max tiles per group / PSUM chain
NCORES = 8
CLASSES = (8, 4, 2, 1)
# groups per DMA chunk / PSUM wave, per class (24 slots = 4 banks, 12 = 2)
CHUNK_GROUPS = {8: 24, 4: 12, 2: 12, 1: 12}  # groups per DMA chunk
PSUM_SLOTS_PER_BANK = 6  # 6 x 80 f32 = 480 of 512
WAVE = 12                # groups per PSUM wave (2 banks)

_BUILD_CACHE = {}


def _frustum():
    ds = np.arange(1.0, 60.0, 0.5, dtype=np.float32)
    xs = np.linspace(0.0, IW - 1.0, FW, dtype=np.float32)
    ys = np.linspace(0.0, IH - 1.0, FH, dtype=np.float32)
    ds_g = np.broadcast_to(ds[:, None, None], (D, FH, FW))
    xs_g = np.broadcast_to(xs[None, None, :], (D, FH, FW))
    ys_g = np.broadcast_to(ys[None, :, None], (D, FH, FW))
    return np.stack([xs_g, ys_g, ds_g], axis=-1)  # [D,FH,FW,3]


def _get_geometry(c2l_rots, c2l_trans, intrins, post_rots, post_trans,
                  extra_rots, extra_trans):
    fr = _frustum()
    pts = fr[None, None] - post_trans[:, :, None, None, None, :]
    inv_pr = np.linalg.inv(post_rots).astype(np.float32)
    pts = np.einsum('bnij,bndhwj->bndhwi', inv_pr, pts).astype(np.float32)
    pts = np.concatenate([pts[..., :2] * pts[..., 2:3], pts[..., 2:3]], axis=-1)
    combine = np.einsum(
        'bnij,bnjk->bnik', c2l_rots, np.linalg.inv(intrins).astype(np.float32)
    ).astype(np.float32)
    pts = np.einsum('bnij,bndhwj->bndhwi', combine, pts).astype(np.float32)
    pts = pts + c2l_trans[:, :, None, None, None, :]
    pts = np.einsum('bij,bndhwj->bndhwi', extra_rots, pts).astype(np.float32)
    pts = pts + extra_trans[:, None, None, None, None, :]
    return pts  # [B,N,D,FH,FW,3]


def _flat_bins(geom):
    """Per-point flat bin id (int64), -1 for dropped points."""
    coords = ((geom - (BXc - DXc / 2.0)) / DXc).astype(np.int32)
    B = coords.shape[0]
    coords = coords.reshape(B, -1, 3)
    cx, cy, cz = coords[..., 0], coords[..., 1], coords[..., 2]
    kept = (cx >= 0) & (cx < NXg) & (cy >= 0) & (cy < NYg) & (cz >= 0) & (cz < NZg)
    flat = ((cz.astype(np.int64) * NXg + cx) * NYg + cy)
    flat = np.where(kept, flat, -1)
    return flat  # [B, Np]


def _round_up(x, m):
    return ((x + m - 1) // m) * m


def _cut_groups(fk_sorted):
    """Greedy: groups of <=KA*128 points spanning < W bins, binary-decomposed
    into class segments [(cls, start, npts, base), ...] in stream order."""
    n = len(fk_sorted)
    segs = []
    i = 0
    while i < n:
        hi = np.searchsorted(fk_sorted, fk_sorted[i] + W, side='left')
        j = min(i + KA * 128, hi, n)
        npts = j - i
        base = int(fk_sorted[i])
        nt = (npts + 127) // 128
        s = i
        for c in CLASSES:
            while nt >= c:
                ln = min(c * 128, j - s)
                segs.append((c, s, ln, base))
                s += ln
                nt -= c
        i = j
    return segs


def _split_classes(segs):
    """Per class: contiguous split across cores balanced by group count,
    padded to uniform per-class counts. {cls: (percore seg lists, Gmax)}."""
    out = {}
    for c in CLASSES:
        cl = [s for s in segs if s[0] == c]
        G = len(cl)
        per = []
        for ci in range(NCORES):
            lo = (G * ci) // NCORES
            hi = (G * (ci + 1)) // NCORES
            per.append(cl[lo:hi])
        Gmax = max(1, max(len(p) for p in per))
        Gmax = _round_up(Gmax, CHUNK_GROUPS[c])
        out[c] = (per, Gmax)
    return out


def _build_core_inputs(class_split, fk_sorted, pidx_sorted, xflat_bf):
    """Build per-core input dict: per class feats + onehot streams."""
    maps = [dict() for _ in range(NCORES)]
    meta = {c: [] for c in CLASSES}  # per class: percore array of bases
    for c in CLASSES:
        per, Gmax = class_split[c]
        T = Gmax * c
        for ci in range(NCORES):
            segs = per[ci]
            feats = np.zeros((T, 128, C), BF16)
            oh = np.zeros((T, 128, W), BF16)
            bases = np.full((Gmax,), -1, np.int64)
            for gi, (_, s, ln, base) in enumerate(segs):
                bases[gi] = base
                lids = (fk_sorted[s:s + ln] - base).astype(np.int64)
                pix = pidx_sorted[s:s + ln]
                t0 = gi * c
                nt = (ln + 127) // 128
                for k in range(nt):
                    a, b = k * 128, min((k + 1) * 128, ln)
                    m = b - a
                    feats[t0 + k, :m] = xflat_bf[pix[a:b]]
                    oh[t0 + k, np.arange(m), lids[a:b]] = 1
            CH = CHUNK_GROUPS[c]
            nch = Gmax // CH
            f = feats.reshape(nch, CH * c, 128, C).transpose(0, 2, 1, 3) \
                     .reshape(nch, 128, CH * c * C)
            o8 = oh.astype(ml_dtypes.float8_e4m3)
            o = o8.reshape(nch, CH * c, 128, W).transpose(0, 2, 1, 3) \
                  .reshape(nch, 128, CH * c * W)
            maps[ci][f"feats{c}"] = np.ascontiguousarray(f)
            maps[ci][f"oh{c}"] = np.ascontiguousarray(o)
            meta[c].append(bases)
    return maps, meta


def _build_bass(shape_key):
    """shape_key: tuple of (cls, Gmax) pairs."""
    if shape_key in _BUILD_CACHE:
        return _BUILD_CACHE[shape_key]
    from concourse import bass, mybir, tile, bacc

    nc = bacc.Bacc()
    params = {}
    for c, Gmax in shape_key:
        CH = CHUNK_GROUPS[c]
        nch = Gmax // CH
        params[f"feats{c}"] = nc.declare_dram_parameter(
            f"feats{c}", [nch, 128, CH * c * C], mybir.dt.bfloat16,
            isOutput=False)
        params[f"oh{c}"] = nc.declare_dram_parameter(
            f"oh{c}", [nch, 128, CH * c * W], mybir.dt.float8e4,
            isOutput=False)
        params[f"out{c}"] = nc.declare_dram_parameter(
            f"out{c}", [W, Gmax, C], mybir.dt.float32, isOutput=True)

    def slot_off(s):
        return (s // PSUM_SLOTS_PER_BANK) * 512 + (s % PSUM_SLOTS_PER_BANK) * C

    # interleave class chunks so short small-class pipelines hide under the
    # dense class-8 stream
    chunk_order = []
    for c, Gmax in shape_key:
        nch = Gmax // CHUNK_GROUPS[c]
        for ch in range(nch):
            chunk_order.append((c, Gmax, ch, (ch + 0.5) / nch))
    chunk_order.sort(key=lambda t: t[3])

    with tile.TileContext(nc) as tc:
        with tc.tile_pool(name="fstream", bufs=5) as fpool, \
             tc.tile_pool(name="stage", bufs=8) as spool, \
             tc.tile_pool(name="psum", bufs=4, space="PSUM") as psum_pool:
            for c, Gmax, ch, _frac in chunk_order:
                CH = CHUNK_GROUPS[c]
                nch = Gmax // CH
                fchunk = fpool.tile([128, CH * c * C], mybir.dt.bfloat16,
                                    tag="fchunk")
                nc.sync.dma_start(fchunk[:], params[f"feats{c}"][ch, :, :])
                ochunk = fpool.tile([128, CH * c * W], mybir.dt.float8e4,
                                    tag="ochunk")
                nc.scalar.dma_start(ochunk[:], params[f"oh{c}"][ch, :, :])
                nwave = (CH + WAVE - 1) // WAVE
                for wv in range(nwave):
                    g0 = wv * WAVE
                    NW = min(WAVE, CH - g0)
                    nbank = NW // PSUM_SLOTS_PER_BANK
                    mega = psum_pool.tile([W, nbank * 512], mybir.dt.float32,
                                          tag="ps")
                    half = NW // 2
                    for gp in range(half):
                        ga, gb = g0 + gp, g0 + gp + half
                        oa, ob = slot_off(gp), slot_off(gp + half)
                        for k in range(c):
                            ta = ga * c + k
                            tb = gb * c + k
                            nc.tensor.matmul(
                                out=mega[:, oa:oa + C],
                                lhsT=ochunk[:, ta * W:(ta + 1) * W],
                                rhs=fchunk[:, ta * C:(ta + 1) * C],
                                start=(k == 0), stop=(k == c - 1))
                            nc.tensor.matmul(
                                out=mega[:, ob:ob + C],
                                lhsT=ochunk[:, tb * W:(tb + 1) * W],
                                rhs=fchunk[:, tb * C:(tb + 1) * C],
                                start=(k == 0), stop=(k == c - 1))
                    st = spool.tile([W, NW, C], mybir.dt.float32, tag="st")
                    src_ap = bass.AP(
                        mega[:].tensor, mega[:].offset,
                        [mega[:].ap[0], [512, nbank],
                         [C, PSUM_SLOTS_PER_BANK], [1, C]])
                    dst_ap = bass.AP(
                        st[:].tensor, st[:].offset,
                        [st[:].ap[0], [PSUM_SLOTS_PER_BANK * C, nbank],
                         [C, PSUM_SLOTS_PER_BANK], [1, C]])
                    nc.scalar.copy(dst_ap, src_ap)
                    nc.scalar.dma_start(
                        params[f"out{c}"][:, ch * CH + g0:ch * CH + g0 + NW, :],
                        st[:])
    nc.finalize()
    _BUILD_CACHE[shape_key] = nc
    return nc


def run_scheduled(x, flat, trace=False, trace_cores=None):
    """Core pipeline given precomputed flat bins; returns (grid, results)."""
    from concourse.bass_utils import run_bass_kernel_spmd

    xflat_bf = np.ascontiguousarray(x.reshape(-1, C)).astype(BF16)
    kept_idx = np.nonzero(flat >= 0)[0]
    fk = flat[kept_idx]
    order = np.argsort(fk, kind='stable')
    fk_sorted = fk[order]
    pidx_sorted = kept_idx[order]

    segs = _cut_groups(fk_sorted)
    class_split = _split_classes(segs)
    shape_key = tuple((c, class_split[c][1]) for c in CLASSES)

    maps, meta = _build_core_inputs(class_split, fk_sorted, pidx_sorted,
                                    xflat_bf)
    nc = _build_bass(shape_key)
    res = run_bass_kernel_spmd(nc, maps, core_ids=list(range(NCORES)),
                               trace=trace, trace_cores=trace_cores)

    grid = np.zeros((NBINS + W, C), np.float32)
    for c in CLASSES:
        for ci in range(NCORES):
            outs = res.results[ci][f"out{c}"]   # [W, Gmax, C]
            bases = meta[c][ci]
            for gi in range(len(bases)):
                base = bases[gi]
                if base >= 0:
                    grid[base:base + W] += outs[:, gi]
    return grid[:NBINS], res


def kernel(x, camera2lidar_rots, camera2lidar_trans, intrins, post_rots,
           post_trans, extra_rots, extra_trans):
    x = np.asarray(x, np.float32)
    B, N = x.shape[0], x.shape[1]
    assert (B, N) == (1, 6) and x.shape[2:] == (D, FH, FW, C), x.shape

    geom = _get_geometry(
        np.asarray(camera2lidar_rots, np.float32),
        np.asarray(camera2lidar_trans, np.float32),
        np.asarray(intrins, np.float32),
        np.asarray(post_rots, np.float32),
        np.asarray(post_trans, np.float32),
        np.asarray(extra_rots, np.float32),
        np.asarray(extra_trans, np.float32),
    )
    flat = _flat_bins(geom)[0]          # [Np]
    grid, _ = run_scheduled(x, flat)
    outp = grid.reshape(NXg, NYg, C).transpose(2, 0, 1)[None]  # [1,C,NX,NY]
    return np.ascontiguousarray(outp)
